# revision 28
# baseline (speedup 1.0000x reference)
"""Ring-attention (context-parallel) kernel for 8 TRN2 NeuronCores.

Problem: x_q [1,2048,2048], x_kv [1,8192,2048], GQA attention (16 q heads,
4 kv heads, D=128) where q occupies global positions 6144..8191 of the
8192-long key sequence (causal on the last 2048 block, full attention on
the first 6144 keys), followed by an output projection.

Strategy (sequence/context parallel, the module's native layout):
  - q rows are split into 16 strips of 128; core c owns strips {c, 15-c}
    (folded pairing -> every core attends to the same total number of keys,
    perfectly balancing the causal wedge).
  - x_kv is sequence-sharded 8 x 1024 rows; each core projects its local
    K/V shard to K^T / V (bf16), then one AllGather over the 8 cores shares
    the full K^T / V (the "ring" collapsed into a single on-chip collective;
    compute overlaps with it).
  - Projection weights are replicated (bf16).
  - Each core computes all 16 heads for its 256 q rows, then the full
    output projection for those rows -> no cross-core reduction at the end.

All host-side tensors are pre-transposed so every on-device matmul consumes
natural layouts (contraction on the partition dim):
  S^T[k,q] = (K^T)lhsT . (Q^T)rhs, softmax along partitions via chunked
  VectorE accumulation + ones-matmul reduction, normalization broadcast via
  a rank-1 ones-outer-product on TensorE, and out^T = (Wo^T)lhsT . (AO^T).
"""

import numpy as np
import ml_dtypes

import concourse.bass as bass
import concourse.mybir as mybir
import concourse.tile as tile
from concourse import bacc, bass_utils

BF16 = ml_dtypes.bfloat16
F32 = mybir.dt.float32
BF = mybir.dt.bfloat16

N_CORES = 8
H = 16          # query heads
HKV = 4         # kv heads
D = 128         # head dim
HID = H * D     # 2048
SL = 2048       # q rows (global)
SKV = 8192      # kv rows (global)
QS = 256        # q rows per core (2 strips of 128)
LKV = SKV // N_CORES   # 1024 local kv rows
HC = HID // 128        # 16 hid chunks
KC = SKV // 128        # 64 key chunks
RANK_OFF = SKV - SL    # 6144: global position of q row 0
BND = RANK_OFF // 128  # 48: first key chunk needing a causal mask
SCALE = 1.0 / float(np.sqrt(D))

_CACHE = {}
DEBUG_TAPS = False


def _build():
    nc = bacc.Bacc("TRN2", target_bir_lowering=False, debug=False,
                   num_devices=N_CORES)

    xqT = nc.dram_tensor("xqT", [HID, QS], BF, kind="ExternalInput")
    xkvT = nc.dram_tensor("xkvT", [HID, LKV], BF, kind="ExternalInput")
    wqT = nc.dram_tensor("wqT", [HID, HID], BF, kind="ExternalInput")
    wkT = nc.dram_tensor("wkT", [HID, HKV * D], BF, kind="ExternalInput")
    wvT = nc.dram_tensor("wvT", [HID, HKV * D], BF, kind="ExternalInput")
    woT = nc.dram_tensor("woT", [HID, HID], BF, kind="ExternalInput")
    # boundary causal masks: 16 key chunks (48..63) x [128 keys, 4 heads x 256 q]
    maskD = nc.dram_tensor("mask", [16 * 128, 4 * QS], BF, kind="ExternalInput")
    outT = nc.dram_tensor("outT", [HID, QS], F32, kind="ExternalOutput")
    taps = None
    if DEBUG_TAPS:
        taps = {
            "d_qt": nc.dram_tensor("d_qt", [HID, QS], BF,
                                   kind="ExternalOutput"),
            "d_ktg": nc.dram_tensor("d_ktg", [N_CORES * HKV * D, LKV], BF,
                                    kind="ExternalOutput"),
            "d_vg": nc.dram_tensor("d_vg", [SKV, HKV * D], BF,
                                   kind="ExternalOutput"),
            "d_acc": nc.dram_tensor("d_acc", [128, 4 * QS], F32,
                                    kind="ExternalOutput"),
            "d_ao": nc.dram_tensor("d_ao", [HKV * 128, 4 * QS], BF,
                                   kind="ExternalOutput"),
        }

    with tile.TileContext(nc) as tc:
        _body(nc, tc, xqT, xkvT, wqT, wkT, wvT, woT, maskD, outT, taps)
    nc.compile()
    return nc


def _body(nc, tc, xqT, xkvT, wqT, wkT, wvT, woT, maskD, outT, taps=None):
    from contextlib import ExitStack
    ctx = ExitStack()
    with ctx:
        const = ctx.enter_context(tc.tile_pool(name="const", bufs=1))
        persist = ctx.enter_context(tc.tile_pool(name="persist", bufs=1))
        dram = ctx.enter_context(tc.tile_pool(name="dram", bufs=1, space="DRAM"))

        ones_k = const.tile([128, 1], BF)
        nc.gpsimd.memset(ones_k[:], 1.0)
        ones_1 = const.tile([1, 128], BF)
        nc.gpsimd.memset(ones_1[:], 1.0)

        # resident inputs. DMA issue order matters: Phase A inputs first
        # (xkv/wk/wv, chunked so compute starts as soon as chunk 0 lands),
        # then xq (Phase C), weights, masks.
        qt_sb = persist.tile([128, H, QS], BF)        # Q^T per head
        ao_sb = persist.tile([128, HKV, 4 * QS], BF)  # normalized O^T per g

        # ---------------- Phase A: local K/V projection -------------------
        # per-g AllGather bounce (1D): bytes [0 : 128*LKV) = K^T_g [D, LKV],
        # bytes [128*LKV : 256*LKV) = V_g [LKV, D] row-major
        bnc = [dram.tile([256 * LKV], BF, name=f"bnc{g}", uniquify=False)
               for g in range(HKV)]
        with (
            tc.tile_pool(name="kva", bufs=1) as kva,
            tc.tile_pool(name="kvps", bufs=2, space="PSUM") as kvps,
        ):
            xkv_sb = kva.tile([128, HC, LKV], BF)
            wk_sb = kva.tile([128, HC, HKV * D], BF)
            wv_sb = kva.tile([128, HC, HKV * D], BF)
            for hc in range(HC):
                nc.sync.dma_start(
                    wk_sb[:, hc, :], wkT.ap()[hc * 128:(hc + 1) * 128, :])
                nc.sync.dma_start(
                    xkv_sb[:, hc, :], xkvT.ap()[hc * 128:(hc + 1) * 128, :])
                nc.sync.dma_start(
                    wv_sb[:, hc, :], wvT.ap()[hc * 128:(hc + 1) * 128, :])

            # lower-priority resident loads on the gpsimd (SWDGE) queue so
            # they don't block the critical Phase A stream
            xq_sb = persist.tile([128, HC, QS], BF)   # Q^T input, hid-chunked
            nc.gpsimd.dma_start(
                xq_sb[:], xqT.ap().rearrange("(a p) q -> p a q", p=128))
            mask_sb = persist.tile([128, 16, 4 * QS], BF)
            nc.gpsimd.dma_start(
                mask_sb[:], maskD.ap().rearrange("(a p) q -> p a q", p=128))

            # K^T_g [D, LKV] = sum_hc wkT[hc, g].T @ xkvT[hc]
            for g in range(HKV):
                ps = kvps.tile([128, LKV], F32, tag="kt")
                for hc in range(HC):
                    lhsT = wk_sb[:, hc, g * D:(g + 1) * D]
                    for nn in range(0, LKV, 512):
                        nc.tensor.matmul(
                            ps[:, nn:nn + 512], lhsT,
                            xkv_sb[:, hc, nn:nn + 512],
                            start=(hc == 0), stop=(hc == HC - 1))
                kt_loc = kva.tile([128, LKV], BF, tag="ktloc")
                nc.scalar.activation(
                    kt_loc[:], ps[:], mybir.ActivationFunctionType.Identity)
                nc.gpsimd.dma_start(
                    bnc[g][0:128 * LKV].rearrange("(p c) -> p c", p=128),
                    kt_loc[:])

            # V chunks [128 keys, 512 dv] = sum_hc xkvT[hc, chunk].T @ wvT[hc]
            for lc in range(LKV // 128):
                ps = kvps.tile([128, HKV * D], F32, tag="v")
                for hc in range(HC):
                    nc.tensor.matmul(
                        ps[:], xkv_sb[:, hc, lc * 128:(lc + 1) * 128],
                        wv_sb[:, hc, :],
                        start=(hc == 0), stop=(hc == HC - 1))
                v_loc = kva.tile([128, HKV * D], BF, tag="vloc")
                nc.scalar.activation(
                    v_loc[:], ps[:], mybir.ActivationFunctionType.Identity)
                # scatter the 4 per-g column blocks into the per-g bounce
                # tiles; V region is [LKV, D] row-major
                for g in range(HKV):
                    nc.gpsimd.dma_start(
                        bnc[g][128 * LKV + lc * 128 * D:
                               128 * LKV + (lc + 1) * 128 * D]
                        .rearrange("(p d) -> p d", p=128),
                        v_loc[:, g * D:(g + 1) * D])

        # ------- Phase B: per-head-group AllGather (pipelines with D) -----
        rg = [list(range(N_CORES))]
        gath = []
        for g in range(HKV):
            gg = dram.tile([N_CORES * 256 * LKV], BF, addr_space="Shared",
                           name=f"gath{g}", uniquify=False)
            nc.gpsimd.collective_compute(
                "AllGather", mybir.AluOpType.bypass, replica_groups=rg,
                ins=[bnc[g].opt()], outs=[gg.opt()])
            gath.append(gg)

        # ---------------- Phase C: Q projection (overlaps B) --------------
        with (
            tc.tile_pool(name="qw", bufs=12) as qw,
            tc.tile_pool(name="qps", bufs=2, space="PSUM") as qps,
        ):
            for hb in range(4):            # blocks of 4 heads
                # one PSUM bank (512 f32) per head: interleaved accumulation
                # groups may not share a 2KB zero region
                ps = qps.tile([128, 4, 512], F32, tag="q")
                for hc in range(HC):
                    w_t = qw.tile([128, 512], BF, tag="wq")
                    nc.sync.dma_start(
                        w_t[:],
                        wqT.ap()[hc * 128:(hc + 1) * 128,
                                 hb * 512:(hb + 1) * 512])
                    for hh in range(4):
                        nc.tensor.matmul(
                            ps[:, hh, 0:QS],
                            w_t[:, hh * 128:(hh + 1) * 128],
                            xq_sb[:, hc, :],
                            start=(hc == 0), stop=(hc == HC - 1))
                for hh in range(4):
                    nc.scalar.activation(
                        qt_sb[:, hb * 4 + hh, :], ps[:, hh, 0:QS],
                        mybir.ActivationFunctionType.Identity)
            if taps is not None:
                nc.sync.dma_start(
                    taps["d_qt"].ap().rearrange("(a p) q -> p a q", p=128),
                    qt_sb[:])

        # ---------------- Phase D: attention ------------------------------
        with (
            tc.tile_pool(name="kvstream", bufs=3) as kvstream,
            tc.tile_pool(name="attw", bufs=3) as attw,
            tc.tile_pool(name="wop", bufs=2) as wop,
            tc.tile_pool(name="accp", bufs=1) as accp,
            tc.tile_pool(name="stps", bufs=2, space="PSUM") as stps,
            tc.tile_pool(name="otps", bufs=1, space="PSUM") as otps,
            tc.tile_pool(name="finps", bufs=1, space="PSUM") as finps,
            tc.tile_pool(name="fps", bufs=1, space="PSUM") as fps,
        ):
            out_acc = persist.tile([128, HC, QS], F32)
            W = 4 * QS            # 1024: 4 heads x 256 q
            for g in range(HKV):
                ot_ps = otps.tile([128, W], F32, tag="ot")
                # bf16 denominator accumulator, 2 chunk-halves wide (the two
                # halves are summed exactly in the f32 ones-matmul below)
                acc2 = accp.tile([128, 2 * W], BF, tag="acc")
                for r in range(N_CORES):
                    # stream rank r's K^T / V slabs for this head group
                    base = r * 256 * LKV
                    kt_slab = kvstream.tile([128, LKV], BF, tag="kt")
                    nc.sync.dma_start(
                        kt_slab[:],
                        gath[g][base:base + 128 * LKV]
                        .rearrange("(p c) -> p c", p=128))
                    v_slab = kvstream.tile([128, LKV // 128, D], BF, tag="v")
                    nc.sync.dma_start(
                        v_slab[:],
                        gath[g][base + 128 * LKV:base + 256 * LKV]
                        .rearrange("(a p d) -> p a d", p=128, d=D))
                    for l2 in range(LKV // 256):      # pairs of key chunks
                        ex = attw.tile([128, 2 * W], BF, tag="ex")
                        for eps in range(2):
                            l = l2 * 2 + eps
                            kc = r * (LKV // 128) + l
                            st = stps.tile([128, W], F32, tag="st")
                            exh = ex[:, eps * W:(eps + 1) * W]
                            if kc >= 56:
                                # strip-0 q cols are fully causal-masked for
                                # every core here: compute only the high half
                                # (the mask mul below zeroes the stale half)
                                for hh in range(4):
                                    nc.tensor.matmul(
                                        st[:, hh * QS + 128:(hh + 1) * QS],
                                        kt_slab[:, l * 128:(l + 1) * 128],
                                        qt_sb[:, g * 4 + hh, 128:QS],
                                        start=True, stop=True)
                                nc.scalar.activation(
                                    exh.rearrange(
                                        "p (h q) -> p h q", q=QS)[:, :, 128:],
                                    st[:].rearrange(
                                        "p (h q) -> p h q", q=QS)[:, :, 128:],
                                    mybir.ActivationFunctionType.Exp,
                                    scale=SCALE)
                            else:
                                for hh in range(4):
                                    nc.tensor.matmul(
                                        st[:, hh * QS:(hh + 1) * QS],
                                        kt_slab[:, l * 128:(l + 1) * 128],
                                        qt_sb[:, g * 4 + hh, :],
                                        start=True, stop=True)
                                nc.scalar.activation(
                                    exh, st[:],
                                    mybir.ActivationFunctionType.Exp,
                                    scale=SCALE)
                            if kc >= BND:
                                nc.vector.tensor_mul(
                                    exh, exh, mask_sb[:, kc - BND, :])
                            for nn in range(0, W, 512):
                                if kc >= 56:
                                    # strip-0 cols are zero here: accumulate
                                    # only the live high halves
                                    nc.tensor.matmul(
                                        ot_ps[:, nn:nn + 512].rearrange(
                                            "p (h q) -> p h q",
                                            q=QS)[:, :, 128:],
                                        v_slab[:, l, :],
                                        ex[:, eps * W + nn:
                                           eps * W + nn + 512].rearrange(
                                            "p (h q) -> p h q",
                                            q=QS)[:, :, 128:],
                                        start=(kc == 0),
                                        stop=(kc == KC - 1))
                                else:
                                    nc.tensor.matmul(
                                        ot_ps[:, nn:nn + 512],
                                        v_slab[:, l, :],
                                        ex[:, eps * W + nn:
                                           eps * W + nn + 512],
                                        start=(kc == 0), stop=(kc == KC - 1))
                        if r == 0 and l2 == 0:
                            nc.vector.tensor_copy(acc2[:], ex[:])
                        else:
                            nc.vector.tensor_add(acc2[:], acc2[:], ex[:])

                # copy attention output out unnormalized first, so ot_ps frees
                # for the next head group while the normalization tail runs
                nc.vector.tensor_copy(ao_sb[:, g, :], ot_ps[:])
                if taps is not None and g == 0:
                    nc.sync.dma_start(taps["d_acc"].ap(), acc2[:, 0:W])
                for nn in range(0, W, 512):
                    den = finps.tile([1, 512], F32, tag="fin")
                    nc.tensor.matmul(den[:], ones_k[:],
                                     acc2[:, nn:nn + 512],
                                     start=True, stop=False)
                    nc.tensor.matmul(den[:], ones_k[:],
                                     acc2[:, W + nn:W + nn + 512],
                                     start=False, stop=True)
                    recip_f = attw.tile([1, 512], F32, tag="recipf")
                    nc.vector.reciprocal(recip_f[:], den[:])
                    recip = attw.tile([1, 512], BF, tag="recip")
                    nc.vector.tensor_copy(recip[:], recip_f[:])
                    bc = finps.tile([128, 512], F32, tag="fin")
                    nc.tensor.matmul(bc[:], ones_1[:], recip[:],
                                     start=True, stop=True)
                    bc_sb = attw.tile([128, 512], BF, tag="bcsb")
                    nc.vector.tensor_copy(bc_sb[:], bc[:])
                    nc.vector.tensor_mul(ao_sb[:, g, nn:nn + 512],
                                         ao_sb[:, g, nn:nn + 512], bc_sb[:])

                # fold this head group into the output projection now (PE has
                # slack during the next group's attention); out_acc holds the
                # running f32 sum over groups
                wo_g = wop.tile([128, 4, HID], BF, tag="wog")
                nc.sync.dma_start(
                    wo_g[:],
                    woT.ap()[g * 512:(g + 1) * 512, :]
                    .rearrange("(a p) d -> p a d", p=128))
                for jc in range(HC):
                    fp = fps.tile([128, QS], F32, tag="fp")
                    for hh in range(4):
                        nc.tensor.matmul(
                            fp[:], wo_g[:, hh, jc * 128:(jc + 1) * 128],
                            ao_sb[:, g, hh * QS:(hh + 1) * QS],
                            start=(hh == 0), stop=(hh == 3))
                    if g == 0:
                        nc.vector.tensor_copy(out_acc[:, jc, :], fp[:])
                    else:
                        nc.vector.tensor_add(out_acc[:, jc, :],
                                             out_acc[:, jc, :], fp[:])
            if taps is not None:
                nc.sync.dma_start(
                    taps["d_ao"].ap().rearrange("(a p) q -> p a q", p=128),
                    ao_sb[:])

        # ---------------- Phase F: store the accumulated output -----------
        for jc in range(HC):
            nc.sync.dma_start(outT.ap()[jc * 128:(jc + 1) * 128, :],
                              out_acc[:, jc, :])



def _get_nc():
    if "nc" not in _CACHE:
        _CACHE["nc"] = _build()
    return _CACHE["nc"]


def _make_in_maps(x_q, x_kv, Wq, Wk, Wv, Wo):
    xqT_full = np.ascontiguousarray(x_q[0].T)           # [HID, SL]
    xkvT_full = np.ascontiguousarray(x_kv[0].T)         # [HID, SKV]
    wqT = np.ascontiguousarray(Wq.T).astype(BF16)
    wkT = np.ascontiguousarray(Wk.T).astype(BF16)
    wvT = np.ascontiguousarray(Wv.T).astype(BF16)
    woT = np.ascontiguousarray(Wo.T).astype(BF16)

    in_maps = []
    for c in range(N_CORES):
        s0, s1 = c, 15 - c
        xqT = np.concatenate(
            [xqT_full[:, s0 * 128:(s0 + 1) * 128],
             xqT_full[:, s1 * 128:(s1 + 1) * 128]], axis=1).astype(BF16)
        xkvT = np.ascontiguousarray(
            xkvT_full[:, c * LKV:(c + 1) * LKV]).astype(BF16)
        # causal masks for key chunks 48..63, replicated across the 4 heads
        # of a kv group (so one tensor_mul covers [128, 4*QS])
        mask = np.zeros((16, 128, QS), dtype=np.float32)
        kk = np.arange(128)
        for j in range(16):
            key_g = (BND + j) * 128 + kk                # [128]
            for half, st in enumerate((s0, s1)):
                q_g = RANK_OFF + st * 128 + np.arange(128)   # [128]
                mask[j, :, half * 128:(half + 1) * 128] = (
                    key_g[:, None] <= q_g[None, :])
        mask4 = np.tile(mask, (1, 1, 4))                # [16, 128, 4*QS]
        in_maps.append({
            "xqT": xqT, "xkvT": xkvT, "wqT": wqT, "wkT": wkT,
            "wvT": wvT, "woT": woT,
            "mask": mask4.reshape(16 * 128, 4 * QS).astype(BF16),
        })
    return in_maps


def _unshard(results):
    out = np.empty((1, SL, HID), dtype=np.float32)
    for c in range(N_CORES):
        outT = results[c]["outT"]                       # [HID, QS]
        s0, s1 = c, 15 - c
        out[0, s0 * 128:(s0 + 1) * 128, :] = outT[:, 0:128].T
        out[0, s1 * 128:(s1 + 1) * 128, :] = outT[:, 128:256].T
    return out


def kernel(x_q, x_kv, Wq, Wk, Wv, Wo, _trace=False, _result_box=None):
    nc = _get_nc()
    in_maps = _make_in_maps(x_q, x_kv, Wq, Wk, Wv, Wo)
    res = bass_utils.run_bass_kernel_spmd(
        nc, in_maps, core_ids=list(range(N_CORES)), trace=_trace)
    if _result_box is not None:
        _result_box.append(res)
    return _unshard(res.results)


# revision 29
# speedup vs baseline: 1.0198x; 1.0198x over previous
"""Ring-attention (context-parallel) kernel for 8 TRN2 NeuronCores.

Problem: x_q [1,2048,2048], x_kv [1,8192,2048], GQA attention (16 q heads,
4 kv heads, D=128) where q occupies global positions 6144..8191 of the
8192-long key sequence (causal on the last 2048 block, full attention on
the first 6144 keys), followed by an output projection.

Strategy (sequence/context parallel, the module's native layout):
  - q rows are split into 16 strips of 128; core c owns strips {c, 15-c}
    (folded pairing -> every core attends to the same total number of keys,
    perfectly balancing the causal wedge).
  - x_kv is sequence-sharded 8 x 1024 rows; each core projects its local
    K/V shard to K^T / V (bf16), then one AllGather over the 8 cores shares
    the full K^T / V (the "ring" collapsed into a single on-chip collective;
    compute overlaps with it).
  - Projection weights are replicated (bf16).
  - Each core computes all 16 heads for its 256 q rows, then the full
    output projection for those rows -> no cross-core reduction at the end.

All host-side tensors are pre-transposed so every on-device matmul consumes
natural layouts (contraction on the partition dim):
  S^T[k,q] = (K^T)lhsT . (Q^T)rhs, softmax along partitions via chunked
  VectorE accumulation + ones-matmul reduction, normalization broadcast via
  a rank-1 ones-outer-product on TensorE, and out^T = (Wo^T)lhsT . (AO^T).
"""

import numpy as np
import ml_dtypes

import concourse.bass as bass
import concourse.mybir as mybir
import concourse.tile as tile
from concourse import bacc, bass_utils

BF16 = ml_dtypes.bfloat16
F32 = mybir.dt.float32
BF = mybir.dt.bfloat16

N_CORES = 8
H = 16          # query heads
HKV = 4         # kv heads
D = 128         # head dim
HID = H * D     # 2048
SL = 2048       # q rows (global)
SKV = 8192      # kv rows (global)
QS = 256        # q rows per core (2 strips of 128)
LKV = SKV // N_CORES   # 1024 local kv rows
HC = HID // 128        # 16 hid chunks
KC = SKV // 128        # 64 key chunks
RANK_OFF = SKV - SL    # 6144: global position of q row 0
BND = RANK_OFF // 128  # 48: first key chunk needing a causal mask
SCALE = 1.0 / float(np.sqrt(D))

_CACHE = {}
DEBUG_TAPS = False


def _build():
    nc = bacc.Bacc("TRN2", target_bir_lowering=False, debug=False,
                   num_devices=N_CORES)

    xqT = nc.dram_tensor("xqT", [HID, QS], BF, kind="ExternalInput")
    xkvT = nc.dram_tensor("xkvT", [HID, LKV], BF, kind="ExternalInput")
    wqT = nc.dram_tensor("wqT", [HID, HID], BF, kind="ExternalInput")
    wkT = nc.dram_tensor("wkT", [HID, HKV * D], BF, kind="ExternalInput")
    wvT = nc.dram_tensor("wvT", [HID, HKV * D], BF, kind="ExternalInput")
    woT = nc.dram_tensor("woT", [HID, HID], BF, kind="ExternalInput")
    # boundary causal masks: 16 key chunks (48..63) x [128 keys, 4 heads x 256 q]
    maskD = nc.dram_tensor("mask", [16 * 128, 4 * QS], BF, kind="ExternalInput")
    outT = nc.dram_tensor("outT", [HID, QS], F32, kind="ExternalOutput")
    taps = None
    if DEBUG_TAPS:
        taps = {
            "d_qt": nc.dram_tensor("d_qt", [HID, QS], BF,
                                   kind="ExternalOutput"),
            "d_ktg": nc.dram_tensor("d_ktg", [N_CORES * HKV * D, LKV], BF,
                                    kind="ExternalOutput"),
            "d_vg": nc.dram_tensor("d_vg", [SKV, HKV * D], BF,
                                   kind="ExternalOutput"),
            "d_acc": nc.dram_tensor("d_acc", [128, 4 * QS], F32,
                                    kind="ExternalOutput"),
            "d_ao": nc.dram_tensor("d_ao", [HKV * 128, 4 * QS], BF,
                                   kind="ExternalOutput"),
        }

    with tile.TileContext(nc) as tc:
        _body(nc, tc, xqT, xkvT, wqT, wkT, wvT, woT, maskD, outT, taps)
    nc.compile()
    return nc


def _body(nc, tc, xqT, xkvT, wqT, wkT, wvT, woT, maskD, outT, taps=None):
    from contextlib import ExitStack
    ctx = ExitStack()
    with ctx:
        const = ctx.enter_context(tc.tile_pool(name="const", bufs=1))
        persist = ctx.enter_context(tc.tile_pool(name="persist", bufs=1))
        dram = ctx.enter_context(tc.tile_pool(name="dram", bufs=1, space="DRAM"))

        ones_k = const.tile([128, 1], BF)
        nc.gpsimd.memset(ones_k[:], 1.0)
        ones_1 = const.tile([1, 128], BF)
        nc.gpsimd.memset(ones_1[:], 1.0)

        # resident inputs. DMA issue order matters: Phase A inputs first
        # (xkv/wk/wv, chunked so compute starts as soon as chunk 0 lands),
        # then xq (Phase C), weights, masks.
        qt_sb = persist.tile([128, H, QS], BF)        # Q^T per head
        ao_sb = persist.tile([128, HKV, 4 * QS], BF)  # normalized O^T per g

        # ---------------- Phase A: local K/V projection -------------------
        # per-g AllGather bounce (1D): bytes [0 : 128*LKV) = K^T_g [D, LKV],
        # bytes [128*LKV : 256*LKV) = V_g [LKV, D] row-major
        bnc = [dram.tile([256 * LKV], BF, name=f"bnc{g}", uniquify=False)
               for g in range(HKV)]
        with (
            tc.tile_pool(name="kva", bufs=1) as kva,
            tc.tile_pool(name="kvps", bufs=2, space="PSUM") as kvps,
        ):
            xkv_sb = kva.tile([128, HC, LKV], BF)
            wk_sb = kva.tile([128, HC, HKV * D], BF)
            wv_sb = kva.tile([128, HC, HKV * D], BF)
            for hc in range(HC):
                nc.sync.dma_start(
                    wk_sb[:, hc, :], wkT.ap()[hc * 128:(hc + 1) * 128, :])
                nc.sync.dma_start(
                    xkv_sb[:, hc, :], xkvT.ap()[hc * 128:(hc + 1) * 128, :])
                nc.sync.dma_start(
                    wv_sb[:, hc, :], wvT.ap()[hc * 128:(hc + 1) * 128, :])

            # lower-priority resident loads on the gpsimd (SWDGE) queue so
            # they don't block the critical Phase A stream
            xq_sb = persist.tile([128, HC, QS], BF)   # Q^T input, hid-chunked
            nc.gpsimd.dma_start(
                xq_sb[:], xqT.ap().rearrange("(a p) q -> p a q", p=128))
            mask_sb = persist.tile([128, 16, 4 * QS], BF)
            nc.gpsimd.dma_start(
                mask_sb[:], maskD.ap().rearrange("(a p) q -> p a q", p=128))

            # K^T_g [D, LKV] = sum_hc wkT[hc, g].T @ xkvT[hc]
            for g in range(HKV):
                ps = kvps.tile([128, LKV], F32, tag="kt")
                for hc in range(HC):
                    lhsT = wk_sb[:, hc, g * D:(g + 1) * D]
                    for nn in range(0, LKV, 512):
                        nc.tensor.matmul(
                            ps[:, nn:nn + 512], lhsT,
                            xkv_sb[:, hc, nn:nn + 512],
                            start=(hc == 0), stop=(hc == HC - 1))
                kt_loc = kva.tile([128, LKV], BF, tag="ktloc")
                nc.scalar.activation(
                    kt_loc[:], ps[:], mybir.ActivationFunctionType.Identity)
                nc.gpsimd.dma_start(
                    bnc[g][0:128 * LKV].rearrange("(p c) -> p c", p=128),
                    kt_loc[:])

            # V chunks [128 keys, 512 dv] = sum_hc xkvT[hc, chunk].T @ wvT[hc]
            for lc in range(LKV // 128):
                ps = kvps.tile([128, HKV * D], F32, tag="v")
                for hc in range(HC):
                    nc.tensor.matmul(
                        ps[:], xkv_sb[:, hc, lc * 128:(lc + 1) * 128],
                        wv_sb[:, hc, :],
                        start=(hc == 0), stop=(hc == HC - 1))
                v_loc = kva.tile([128, HKV * D], BF, tag="vloc")
                nc.scalar.activation(
                    v_loc[:], ps[:], mybir.ActivationFunctionType.Identity)
                # scatter the 4 per-g column blocks into the per-g bounce
                # tiles; V region is [LKV, D] row-major
                for g in range(HKV):
                    nc.gpsimd.dma_start(
                        bnc[g][128 * LKV + lc * 128 * D:
                               128 * LKV + (lc + 1) * 128 * D]
                        .rearrange("(p d) -> p d", p=128),
                        v_loc[:, g * D:(g + 1) * D])

        # ------- Phase B: per-head-group AllGather (pipelines with D) -----
        rg = [list(range(N_CORES))]
        gath = []
        for g in range(HKV):
            gg = dram.tile([N_CORES * 256 * LKV], BF, addr_space="Shared",
                           name=f"gath{g}", uniquify=False)
            nc.gpsimd.collective_compute(
                "AllGather", mybir.AluOpType.bypass, replica_groups=rg,
                ins=[bnc[g].opt()], outs=[gg.opt()])
            gath.append(gg)

        # ---------------- Phase C: Q projection (overlaps B) --------------
        with (
            tc.tile_pool(name="qw", bufs=12) as qw,
            tc.tile_pool(name="qps", bufs=2, space="PSUM") as qps,
        ):
            for hb in range(4):            # blocks of 4 heads
                # one PSUM bank (512 f32) per head: interleaved accumulation
                # groups may not share a 2KB zero region
                ps = qps.tile([128, 4, 512], F32, tag="q")
                for hc in range(HC):
                    w_t = qw.tile([128, 512], BF, tag="wq")
                    nc.sync.dma_start(
                        w_t[:],
                        wqT.ap()[hc * 128:(hc + 1) * 128,
                                 hb * 512:(hb + 1) * 512])
                    for hh in range(4):
                        nc.tensor.matmul(
                            ps[:, hh, 0:QS],
                            w_t[:, hh * 128:(hh + 1) * 128],
                            xq_sb[:, hc, :],
                            start=(hc == 0), stop=(hc == HC - 1))
                for hh in range(4):
                    nc.scalar.activation(
                        qt_sb[:, hb * 4 + hh, :], ps[:, hh, 0:QS],
                        mybir.ActivationFunctionType.Identity)
            if taps is not None:
                nc.sync.dma_start(
                    taps["d_qt"].ap().rearrange("(a p) q -> p a q", p=128),
                    qt_sb[:])

        # ---------------- Phase D: attention ------------------------------
        with (
            tc.tile_pool(name="kvstream", bufs=3) as kvstream,
            tc.tile_pool(name="attw", bufs=3) as attw,
            tc.tile_pool(name="wop", bufs=2) as wop,
            tc.tile_pool(name="accp", bufs=1) as accp,
            tc.tile_pool(name="stps", bufs=2, space="PSUM") as stps,
            tc.tile_pool(name="otps", bufs=1, space="PSUM") as otps,
            tc.tile_pool(name="finps", bufs=1, space="PSUM") as finps,
            tc.tile_pool(name="fps", bufs=1, space="PSUM") as fps,
        ):
            out_acc = persist.tile([128, HC, QS], F32)
            W = 4 * QS            # 1024: 4 heads x 256 q
            for g in range(HKV):
                ot_ps = otps.tile([128, W], F32, tag="ot")
                # bf16 denominator accumulator, 2 chunk-halves wide (the two
                # halves are summed exactly in the f32 ones-matmul below)
                acc2 = accp.tile([128, 2 * W], BF, tag="acc")
                for r in range(N_CORES):
                    # stream rank r's K^T / V slabs for this head group
                    base = r * 256 * LKV
                    kt_slab = kvstream.tile([128, LKV], BF, tag="kt")
                    nc.sync.dma_start(
                        kt_slab[:],
                        gath[g][base:base + 128 * LKV]
                        .rearrange("(p c) -> p c", p=128))
                    v_slab = kvstream.tile([128, LKV // 128, D], BF, tag="v")
                    nc.sync.dma_start(
                        v_slab[:],
                        gath[g][base + 128 * LKV:base + 256 * LKV]
                        .rearrange("(a p d) -> p a d", p=128, d=D))
                    for l2 in range(LKV // 256):      # pairs of key chunks
                        ex = attw.tile([128, 2 * W], BF, tag="ex")
                        for eps in range(2):
                            l = l2 * 2 + eps
                            kc = r * (LKV // 128) + l
                            st = stps.tile([128, W], F32, tag="st")
                            exh = ex[:, eps * W:(eps + 1) * W]
                            if kc >= 56:
                                # strip-0 q cols are fully causal-masked for
                                # every core here: compute only the high half
                                # (the mask mul below zeroes the stale half)
                                for hh in range(4):
                                    nc.tensor.matmul(
                                        st[:, hh * QS + 128:(hh + 1) * QS],
                                        kt_slab[:, l * 128:(l + 1) * 128],
                                        qt_sb[:, g * 4 + hh, 128:QS],
                                        start=True, stop=True)
                                nc.scalar.activation(
                                    exh.rearrange(
                                        "p (h q) -> p h q", q=QS)[:, :, 128:],
                                    st[:].rearrange(
                                        "p (h q) -> p h q", q=QS)[:, :, 128:],
                                    mybir.ActivationFunctionType.Exp,
                                    scale=SCALE)
                            else:
                                for hh in range(4):
                                    nc.tensor.matmul(
                                        st[:, hh * QS:(hh + 1) * QS],
                                        kt_slab[:, l * 128:(l + 1) * 128],
                                        qt_sb[:, g * 4 + hh, :],
                                        start=True, stop=True)
                                nc.scalar.activation(
                                    exh, st[:],
                                    mybir.ActivationFunctionType.Exp,
                                    scale=SCALE)
                            if kc >= BND:
                                nc.vector.tensor_mul(
                                    exh, exh, mask_sb[:, kc - BND, :])
                            for nn in range(0, W, 512):
                                nc.tensor.matmul(
                                    ot_ps[:, nn:nn + 512],
                                    v_slab[:, l, :],
                                    ex[:, eps * W + nn:eps * W + nn + 512],
                                    start=(kc == 0), stop=(kc == KC - 1))
                        if r == 0 and l2 == 0:
                            nc.vector.tensor_copy(acc2[:], ex[:])
                        else:
                            nc.vector.tensor_add(acc2[:], acc2[:], ex[:])

                # copy attention output out unnormalized first, so ot_ps frees
                # for the next head group while the normalization tail runs
                nc.vector.tensor_copy(ao_sb[:, g, :], ot_ps[:])
                if taps is not None and g == 0:
                    nc.sync.dma_start(taps["d_acc"].ap(), acc2[:, 0:W])
                for nn in range(0, W, 512):
                    den = finps.tile([1, 512], F32, tag="fin")
                    nc.tensor.matmul(den[:], ones_k[:],
                                     acc2[:, nn:nn + 512],
                                     start=True, stop=False)
                    nc.tensor.matmul(den[:], ones_k[:],
                                     acc2[:, W + nn:W + nn + 512],
                                     start=False, stop=True)
                    recip_f = attw.tile([1, 512], F32, tag="recipf")
                    nc.vector.reciprocal(recip_f[:], den[:])
                    recip = attw.tile([1, 512], BF, tag="recip")
                    nc.vector.tensor_copy(recip[:], recip_f[:])
                    bc = finps.tile([128, 512], F32, tag="fin")
                    nc.tensor.matmul(bc[:], ones_1[:], recip[:],
                                     start=True, stop=True)
                    bc_sb = attw.tile([128, 512], BF, tag="bcsb")
                    nc.vector.tensor_copy(bc_sb[:], bc[:])
                    nc.vector.tensor_mul(ao_sb[:, g, nn:nn + 512],
                                         ao_sb[:, g, nn:nn + 512], bc_sb[:])

                # fold this head group into the output projection now (PE has
                # slack during the next group's attention); out_acc holds the
                # running f32 sum over groups
                wo_g = wop.tile([128, 4, HID], BF, tag="wog")
                nc.sync.dma_start(
                    wo_g[:],
                    woT.ap()[g * 512:(g + 1) * 512, :]
                    .rearrange("(a p) d -> p a d", p=128))
                for jc in range(HC):
                    fp = fps.tile([128, QS], F32, tag="fp")
                    for hh in range(4):
                        nc.tensor.matmul(
                            fp[:], wo_g[:, hh, jc * 128:(jc + 1) * 128],
                            ao_sb[:, g, hh * QS:(hh + 1) * QS],
                            start=(hh == 0), stop=(hh == 3))
                    if g == 0:
                        nc.vector.tensor_copy(out_acc[:, jc, :], fp[:])
                    else:
                        nc.vector.tensor_add(out_acc[:, jc, :],
                                             out_acc[:, jc, :], fp[:])
            if taps is not None:
                nc.sync.dma_start(
                    taps["d_ao"].ap().rearrange("(a p) q -> p a q", p=128),
                    ao_sb[:])

        # ---------------- Phase F: store the accumulated output -----------
        for jc in range(HC):
            nc.sync.dma_start(outT.ap()[jc * 128:(jc + 1) * 128, :],
                              out_acc[:, jc, :])



def _get_nc():
    if "nc" not in _CACHE:
        _CACHE["nc"] = _build()
    return _CACHE["nc"]


def _make_in_maps(x_q, x_kv, Wq, Wk, Wv, Wo):
    xqT_full = np.ascontiguousarray(x_q[0].T)           # [HID, SL]
    xkvT_full = np.ascontiguousarray(x_kv[0].T)         # [HID, SKV]
    wqT = np.ascontiguousarray(Wq.T).astype(BF16)
    wkT = np.ascontiguousarray(Wk.T).astype(BF16)
    wvT = np.ascontiguousarray(Wv.T).astype(BF16)
    woT = np.ascontiguousarray(Wo.T).astype(BF16)

    in_maps = []
    for c in range(N_CORES):
        s0, s1 = c, 15 - c
        xqT = np.concatenate(
            [xqT_full[:, s0 * 128:(s0 + 1) * 128],
             xqT_full[:, s1 * 128:(s1 + 1) * 128]], axis=1).astype(BF16)
        xkvT = np.ascontiguousarray(
            xkvT_full[:, c * LKV:(c + 1) * LKV]).astype(BF16)
        # causal masks for key chunks 48..63, replicated across the 4 heads
        # of a kv group (so one tensor_mul covers [128, 4*QS])
        mask = np.zeros((16, 128, QS), dtype=np.float32)
        kk = np.arange(128)
        for j in range(16):
            key_g = (BND + j) * 128 + kk                # [128]
            for half, st in enumerate((s0, s1)):
                q_g = RANK_OFF + st * 128 + np.arange(128)   # [128]
                mask[j, :, half * 128:(half + 1) * 128] = (
                    key_g[:, None] <= q_g[None, :])
        mask4 = np.tile(mask, (1, 1, 4))                # [16, 128, 4*QS]
        in_maps.append({
            "xqT": xqT, "xkvT": xkvT, "wqT": wqT, "wkT": wkT,
            "wvT": wvT, "woT": woT,
            "mask": mask4.reshape(16 * 128, 4 * QS).astype(BF16),
        })
    return in_maps


def _unshard(results):
    out = np.empty((1, SL, HID), dtype=np.float32)
    for c in range(N_CORES):
        outT = results[c]["outT"]                       # [HID, QS]
        s0, s1 = c, 15 - c
        out[0, s0 * 128:(s0 + 1) * 128, :] = outT[:, 0:128].T
        out[0, s1 * 128:(s1 + 1) * 128, :] = outT[:, 128:256].T
    return out


def kernel(x_q, x_kv, Wq, Wk, Wv, Wo, _trace=False, _result_box=None):
    nc = _get_nc()
    in_maps = _make_in_maps(x_q, x_kv, Wq, Wk, Wv, Wo)
    res = bass_utils.run_bass_kernel_spmd(
        nc, in_maps, core_ids=list(range(N_CORES)), trace=_trace)
    if _result_box is not None:
        _result_box.append(res)
    return _unshard(res.results)
